# revision 1
# baseline (speedup 1.0000x reference)
"""LQLinear (2-bit learned VQ linear) Trainium2 kernel.

Math (Q_T=1): the least-squares basis refit only feeds the *discarded*
buffer update, so the forward output is

    out = x @ wq.T + bias

where wq bucketizes weight into the 4 sorted levels {+-b_small +- b_big}
(b_small, b_big = sorted |basis|), thresholds at midpoints {-b_big, 0, +b_big}.

Device strategy (8 cores, out_features-sharded, 512 rows each):
  - wq = b_small * wqn with wqn in {+-1, +-3} for the reference basis
    (b_big = 2*b_small): EXACT in fp8e4.
  - greedy sign quantization == bucketize, decided in f32 (bf16/fp16
    thresholds would flip ~0.2-0.4% of weights near +-b_big, ~4% out
    err): s_big = sign(w), ss2 = sign(w^2 - b_big^2) == sign(|w|-b_big),
    wqn = s_big * (2 + ss2).  w^2 on DVE so ACT does 2 ops/k-tile.
  - GEMM: stationary wqn fp8 [128k,128o], moving x bf16 [128k,512t]
    (rel err 1.46e-3, all from bf16 rounding of x), fp32 PSUM, 2048
    MMs/core at the warm back-to-back rate ~214ns (N=512 @ 2.4 GHz);
    measured ~477us vs the 437us pure-MM floor, PE busy ~92%.
  - Quantize is pipelined per k-tile; tb=0 consumes k-tiles in bursts
    of 8 across its 4 PSUM banks so the PE works while wq trickles in.
  - w-loads are interleaved ahead of x tb-fetches in groups of 8 on the
    SAME (sync) HW-DGE ring: on separate rings the SDMA engines'
    per-packet round-robin starves the 2KB w packets behind 16KB x
    packets (measured 22-80 GB/s -> quantize crawled at 4-5us/k-tile).
    Out-stores get the scalar ring to stay off the input path.
  - DVE evicts PSUM with fused out = b_small*psum + bias[o].
  - Host prep is layout-only sharding work (transpose/cast/block) so
    every device DMA is contiguous per partition.

Things measured NOT to help: fp8 e4m3 x (rel err 2.3e-2 > 2e-2 gate),
fp8 DoubleRow hi+lo (2048 DR-MMs @ ~241ns > bf16 floor), interleaving
2 token blocks across all 8 banks (bank-cycling slows MM pipelining),
HAM warmup dummy MMs (their active time == the idle they remove).
"""

import os
import sys

for _p in ("/opt/trn_rl_repo", "/root/.axon_site/_ro/trn_rl_repo"):
    if os.path.isdir(_p) and _p not in sys.path:
        sys.path.insert(0, _p)

import numpy as np
import ml_dtypes

N_CORES = 8
TOKENS = 8192
IN_F = 4096
OUT_F = 4096
O_SHARD = OUT_F // N_CORES          # 512 output rows per core
KT = IN_F // 128                    # 32 k-tiles
KH = KT // 2                        # x streamed in half-k chunks of 16
TB = 512                            # token block (psum free dim)
N_TB = TOKENS // TB                 # 16 token blocks
O_SUB = O_SHARD // 128              # 4 output subtiles per core

LAST_RUN_INFO = {}


def _build_nc(b_small: float, b_big: float, wdt_name: str):
    import concourse.mybir as mybir
    import concourse.tile as tile
    from concourse import bacc

    dt = mybir.dt
    Alu = mybir.AluOpType

    R = b_big / b_small
    wdt = getattr(dt, wdt_name)     # lhsT dtype: float8e4 (default) or bfloat16

    nc = bacc.Bacc("TRN2", target_bir_lowering=False,
                   debug=os.environ.get("LQ_DEBUG", "0") == "1")

    # blocked, fully-contiguous-per-partition host layouts
    wT = nc.dram_tensor("wT", [KT, 128, O_SHARD], dt.float32, kind="ExternalInput")
    xh = nc.dram_tensor("xh", [N_TB, 2, 128, KH, TB], dt.bfloat16,
                        kind="ExternalInput")
    bs = nc.dram_tensor("bs", [128, O_SUB], dt.float32, kind="ExternalInput")
    oT = nc.dram_tensor("oT", [N_TB, O_SUB, 128, TB], dt.bfloat16,
                        kind="ExternalOutput")

    wT_r = wT.ap()                  # [kt][128, 512]
    xh_r = xh.ap()                  # [tb][h][128, KH, 512]
    oT_r = oT.ap()                  # [tb][osb][128, 512]

    with tile.TileContext(nc) as tc:
        with (
            tc.tile_pool(name="const", bufs=1) as const,
            tc.tile_pool(name="wq", bufs=1) as wqp,
            tc.tile_pool(name="wload", bufs=8) as wload,
            tc.tile_pool(name="quant", bufs=4) as qp,
            tc.tile_pool(name="xhp", bufs=4) as xhp,
            tc.tile_pool(name="outp", bufs=8) as outp,
            tc.tile_pool(name="psum", bufs=8, space="PSUM") as psp,
        ):
            bias_sb = const.tile([128, O_SUB], dt.float32)
            nc.sync.dma_start(bias_sb[:], bs.ap())
            nbb2 = const.tile([128, 1], dt.float32, tag="nbb2")
            nc.vector.memset(nbb2[:], -float(np.float32(b_big) * np.float32(b_big)))
            rcon = const.tile([128, 1], dt.float32, tag="rcon")
            nc.vector.memset(rcon[:], R)

            # x prefetch for the first token blocks starts immediately,
            # racing the quantize pipeline below
            x_tiles = {}

            def fetch_x(tb, eng=None):
                for h in range(2):
                    x_t = xhp.tile([128, KH, TB], dt.bfloat16, tag=f"xh{h}")
                    (eng or nc.sync).dma_start(x_t[:], xh_r[tb, h])
                    x_tiles[(tb, h)] = x_t

            # ---- quantize weight shard -> wqn {+-1,+-R}, one tile per kt
            # w-loads share the sync ring with x, interleaved ahead of the
            # x tb-fetches in groups of 8 so the 2KB w packets are not
            # starved by the 16KB x packets (SDMA round-robins per packet).
            wq_t = []

            def quantize_w(kt):
                w_t = wload.tile([128, O_SHARD], dt.float32, tag="wl")
                nc.sync.dma_start(w_t[:], wT_r[kt])
                sb = qp.tile([128, O_SHARD], dt.float32, tag="sb")
                av = qp.tile([128, O_SHARD], dt.float32, tag="av")
                # ss2 = sign(|w| - b_big) computed as sign(w^2 - b_big^2)
                # (w^2 on DVE so ACT only does 2 ops per k-tile)
                nc.vector.tensor_tensor(av[:], w_t[:], w_t[:], Alu.mult)
                nc.scalar.sign(sb[:], w_t[:])
                nc.scalar.sign(av[:], av[:], bias=nbb2[:])
                # wqn = s_big * (R + ss2)  in {+-(R-1), +-(R+1)}; the +R
                # alternates ACT/DVE per k-tile to balance both at ~2.5
                # ops/k-tile through the quantize window
                if kt % 2 == 0:
                    nc.vector.tensor_scalar(av[:], av[:], R, None, Alu.add)
                else:
                    nc.scalar.activation(av[:], av[:],
                                         mybir.ActivationFunctionType.Identity,
                                         rcon[:])
                wq = wqp.tile([128, O_SHARD], wdt, tag=f"wq{kt}")
                nc.vector.tensor_tensor(wq[:], sb[:], av[:], Alu.mult)
                wq_t.append(wq)

            for kt in range(8):
                quantize_w(kt)
            fetch_x(0)
            for kt in range(8, 16):
                quantize_w(kt)
            fetch_x(1)
            for kt in range(16, 24):
                quantize_w(kt)
            fetch_x(2)
            for kt in range(24, KT):
                quantize_w(kt)
            fetch_x(3)

            def evict(tb, osb, ps):
                o_t = outp.tile([128, TB], dt.bfloat16, tag="ot")
                # out = b_small * psum + bias  (per-partition bias AP)
                nc.vector.tensor_scalar(o_t[:], ps[:], float(b_small),
                                        bias_sb[:, osb:osb + 1],
                                        Alu.mult, Alu.add)
                nc.scalar.dma_start(oT_r[tb, osb], o_t[:])

            # ---- GEMM  psum[o128, t512] += wqn[k,o].T @ xT[k,t]
            # tb=0 runs kt-bursts of 8 across its 4 groups so the PE keeps
            # working as k-tiles emerge from the quantize pipeline instead
            # of FIFO-stalling behind one group's next wq tile.
            xts0 = (x_tiles.pop((0, 0)), x_tiles.pop((0, 1)))
            ps0 = [psp.tile([128, TB], dt.float32, tag="ps", name=f"ps0{osb}")
                   for osb in range(O_SUB)]
            for b in range(KT // 8):
                for osb in range(O_SUB):
                    for kt in range(8 * b, 8 * b + 8):
                        nc.tensor.matmul(
                            ps0[osb][:],
                            wq_t[kt][:, osb * 128:(osb + 1) * 128],
                            xts0[kt // KH][:, kt % KH, :],
                            start=(kt == 0), stop=(kt == KT - 1))
            for osb in range(O_SUB):
                evict(0, osb, ps0[osb])

            for tb in range(1, N_TB):
                if tb + 2 < N_TB:
                    fetch_x(tb + 2)
                xts = (x_tiles.pop((tb, 0)), x_tiles.pop((tb, 1)))
                for osb in range(O_SUB):
                    ps = psp.tile([128, TB], dt.float32, tag="ps", name="ps")
                    for kt in range(KT):
                        nc.tensor.matmul(
                            ps[:],
                            wq_t[kt][:, osb * 128:(osb + 1) * 128],
                            xts[kt // KH][:, kt % KH, :],
                            start=(kt == 0), stop=(kt == KT - 1))
                    if tb == N_TB - 1:
                        # last tb: half-column evict/store slices so the
                        # stores overlap the evictions in the drain tail
                        o_t = outp.tile([128, TB], dt.bfloat16, tag="ot",
                                        name="ot_tail")
                        for half in range(2):
                            sl = slice(half * (TB // 2), (half + 1) * (TB // 2))
                            nc.vector.tensor_scalar(
                                o_t[:, sl], ps[:, sl], float(b_small),
                                bias_sb[:, osb:osb + 1], Alu.mult, Alu.add)
                            nc.scalar.dma_start(oT_r[tb, osb][:, sl],
                                                o_t[:, sl])
                    else:
                        evict(tb, osb, ps)

    nc.compile()
    return nc


def kernel(x, weight, bias, basis):
    from concourse import bass_utils

    x = np.asarray(x, dtype=np.float32)
    weight = np.asarray(weight, dtype=np.float32)
    bias = np.asarray(bias, dtype=np.float32)
    basis = np.asarray(basis, dtype=np.float32)

    b_small, b_big = sorted(float(v) for v in np.abs(basis))
    wdt_name = os.environ.get("LQ_WDT", "float8e4")

    # ---- host-side shard/layout prep (transpose, cast, block)
    # xb[tb, h, p, kt, t] = x[tb*512+t, (h*16+kt)*128+p]
    xb = np.ascontiguousarray(
        x.T.reshape(2, KH, 128, N_TB, TB).transpose(3, 0, 2, 1, 4)
    ).astype(ml_dtypes.bfloat16)
    wt = weight.T                                        # [4096 in, 4096 out]

    in_maps = []
    for c in range(N_CORES):
        wb = np.ascontiguousarray(
            wt[:, c * O_SHARD:(c + 1) * O_SHARD]).reshape(KT, 128, O_SHARD)
        m = {
            "wT": wb,
            "xh": xb,
            "bs": np.ascontiguousarray(
                bias[c * O_SHARD:(c + 1) * O_SHARD].reshape(O_SUB, 128).T),
        }
        in_maps.append(m)

    nc = _build_nc(b_small, b_big, wdt_name)
    trace = os.environ.get("LQ_TRACE", "") == "1"

    # random-projection ground truth for readback validation: catches
    # rare transport-level corruption (e.g. output blocks read back
    # before the final stores land); the device result is still what we
    # return -- this only decides whether to retry the execution.
    rng = np.random.default_rng(12345)
    v = rng.standard_normal(OUT_F)
    wqn = np.sign(weight) * (2.0 + np.sign(weight * weight
                                           - np.float32(b_big) ** 2))
    u = (b_small * wqn.astype(np.float64)).T @ v          # [IN_F]
    r_exp = x.astype(np.float64) @ u + float(bias @ v)    # [TOKENS]
    r_scale = np.linalg.norm(r_exp)

    out = None
    for attempt in range(3):
        try:
            res = bass_utils.run_bass_kernel_spmd(
                nc, in_maps, core_ids=list(range(N_CORES)), trace=trace)
        except Exception:
            if attempt == 2:
                raise
            continue

        LAST_RUN_INFO.clear()
        LAST_RUN_INFO["exec_time_ns"] = res.exec_time_ns
        LAST_RUN_INFO["profile_json"] = res.profile_json
        LAST_RUN_INFO["nc"] = nc
        LAST_RUN_INFO["in_maps"] = in_maps

        # oT blocked [tb, osb, p, t]: rows osb*128+p of shard, cols tb*512+t
        outT = np.concatenate(
            [res.results[c]["oT"].transpose(1, 2, 0, 3).reshape(O_SHARD, TOKENS)
             for c in range(N_CORES)], axis=0)
        out = np.ascontiguousarray(outT.T).astype(np.float32)

        resid = np.linalg.norm(out.astype(np.float64) @ v - r_exp) / r_scale
        if resid < 2e-2:
            break
    return out



# revision 3
# speedup vs baseline: 1.0080x; 1.0080x over previous
"""LQLinear (2-bit learned VQ linear) Trainium2 kernel.

Math (Q_T=1): the least-squares basis refit only feeds the *discarded*
buffer update, so the forward output is

    out = x @ wq.T + bias

where wq bucketizes weight into the 4 sorted levels {+-b_small +- b_big}
(b_small, b_big = sorted |basis|), thresholds at midpoints {-b_big, 0, +b_big}.

Device strategy (8 cores, out_features-sharded, 512 rows each):
  - wq = b_small * wqn with wqn in {+-1, +-3} for the reference basis
    (b_big = 2*b_small): EXACT in fp8e4.
  - greedy sign quantization == bucketize, decided in f32: s_big = sign(w),
    ss2 = sign(w^2 - b_big^2), wqn = s_big * (2 + ss2).
  - GEMM is mixed-precision along k to break the bf16 1-row/cycle floor:
      * k 0..2559 (10 pairs of 256): x cast to fp8e4m3, wqn fp8e4 stationary
        [128,2,128], DoubleRow MMs (2 fp8 rows/cell/cycle, ~241ns per
        256k x 512t vs 2x214ns in bf16).
      * k 2560..4095 (12 tiles of 128): x in bf16, normal MMs (~214ns).
    Host-measured exact error of this split: 1.835e-2 (gate 2e-2); the
    fp8e4-only version is 2.317e-2 (fails) and e3m4 DoubleRow is rejected
    by walrus checkMatmultPerfMode, so this is the fastest admissible mix.
    MM floor: 64 groups x (10x241 + 12x214) ns ~= 319us vs 437us bf16-only.
  - Quantize is pipelined per k-tile; tb=0 consumes tiles in bursts as they
    emerge so the PE works while wq trickles in. DR pairs are quantized
    first (kt 0..19), matching the DR-phase-first MM order.
  - w-loads interleave ahead of x tb-fetches in groups of 8 on the SAME
    (sync) HW-DGE ring (separate rings starve the 2KB w packets behind the
    big x packets). Out-stores use the scalar ring.
  - DVE evicts PSUM with fused out = b_small*psum + bias[o].
  - Host prep is layout-only sharding work (transpose/cast/block).

Measured NOT to help in prior sessions: fp8 e4m3 for all of x (rel err
2.3e-2 > 2e-2 gate), fp8 DoubleRow hi+lo for all k (2048 DR-MMs @ ~241ns >
bf16 floor), interleaving 2 token blocks across all 8 psum banks, HAM
warmup dummy MMs.
"""

import os
import sys

for _p in ("/opt/trn_rl_repo", "/root/.axon_site/_ro/trn_rl_repo"):
    if os.path.isdir(_p) and _p not in sys.path:
        sys.path.insert(0, _p)

import numpy as np
import ml_dtypes

N_CORES = 8
TOKENS = 8192
IN_F = 4096
OUT_F = 4096
O_SHARD = OUT_F // N_CORES          # 512 output rows per core
KT = IN_F // 128                    # 32 k-tiles
TB = 512                            # token block (psum free dim)
N_TB = TOKENS // TB                 # 16 token blocks
O_SUB = O_SHARD // 128              # 4 output subtiles per core

KDR_PAIRS = int(os.environ.get("LQ_KDR_PAIRS", "10"))  # DoubleRow 256-k pairs
KDR = KDR_PAIRS * 256               # fp8 k-dims (default 2560)
KBF = (IN_F - KDR) // 128           # bf16 128-k tiles (default 12)

LAST_RUN_INFO = {}


def _build_nc(b_small: float, b_big: float):
    import concourse.mybir as mybir
    import concourse.tile as tile
    from concourse import bacc

    dt = mybir.dt
    Alu = mybir.AluOpType
    DR = mybir.MatmulPerfMode.DoubleRow

    R = b_big / b_small

    nc = bacc.Bacc("TRN2", target_bir_lowering=False,
                   debug=os.environ.get("LQ_DEBUG", "0") == "1")

    # blocked, fully-contiguous-per-partition host layouts
    wT = nc.dram_tensor("wT", [KT, 128, O_SHARD], dt.float32, kind="ExternalInput")
    xdr = nc.dram_tensor("xdr", [N_TB, 128, KDR_PAIRS, 2, TB], dt.float8e4,
                         kind="ExternalInput")
    xbf = nc.dram_tensor("xbf", [N_TB, 128, KBF, TB], dt.bfloat16,
                         kind="ExternalInput")
    bs = nc.dram_tensor("bs", [128, O_SUB], dt.float32, kind="ExternalInput")
    oT = nc.dram_tensor("oT", [N_TB, O_SUB, 128, TB], dt.bfloat16,
                        kind="ExternalOutput")

    wT_r = wT.ap()                  # [kt][128, 512]
    xdr_r = xdr.ap()                # [tb][128, PAIRS, 2, 512]
    xbf_r = xbf.ap()                # [tb][128, KBF, 512]
    oT_r = oT.ap()                  # [tb][osb][128, 512]

    with tile.TileContext(nc) as tc:
        with (
            tc.tile_pool(name="const", bufs=1) as const,
            tc.tile_pool(name="wq", bufs=1) as wqp,
            tc.tile_pool(name="wload", bufs=8) as wload,
            tc.tile_pool(name="quant", bufs=4) as qp,
            tc.tile_pool(name="xdrp", bufs=3) as xdrp,
            tc.tile_pool(name="xbfp", bufs=3) as xbfp,
            tc.tile_pool(name="outp", bufs=8) as outp,
            tc.tile_pool(name="psum", bufs=8, space="PSUM") as psp,
        ):
            bias_sb = const.tile([128, O_SUB], dt.float32)
            nc.sync.dma_start(bias_sb[:], bs.ap())
            nbb2 = const.tile([128, 1], dt.float32, tag="nbb2")
            nc.vector.memset(nbb2[:], -float(np.float32(b_big) * np.float32(b_big)))
            rcon = const.tile([128, 1], dt.float32, tag="rcon")
            nc.vector.memset(rcon[:], R)

            # persistent quantized-weight tiles: DR pairs + bf16-region tiles
            wq_dr = [wqp.tile([128, 2, O_SHARD], dt.float8e4, tag=f"wqdr{j}",
                              name=f"wqdr{j}")
                     for j in range(KDR_PAIRS)]
            wq_bf = [wqp.tile([128, O_SHARD], dt.float8e4, tag=f"wqbf{c}",
                              name=f"wqbf{c}")
                     for c in range(KBF)]

            x_tiles = {}

            def fetch_x(tb):
                xd = xdrp.tile([128, KDR_PAIRS, 2, TB], dt.float8e4, tag="xd")
                nc.sync.dma_start(xd[:], xdr_r[tb])
                xb = xbfp.tile([128, KBF, TB], dt.bfloat16, tag="xb")
                nc.sync.dma_start(xb[:], xbf_r[tb])
                x_tiles[tb] = (xd, xb)

            # ---- quantize weight shard -> wqn {+-1,+-R} fp8, one tile per kt
            def quantize_w(kt):
                w_t = wload.tile([128, O_SHARD], dt.float32, tag="wl")
                nc.sync.dma_start(w_t[:], wT_r[kt])
                sb = qp.tile([128, O_SHARD], dt.float32, tag="sb")
                av = qp.tile([128, O_SHARD], dt.float32, tag="av")
                # ss2 = sign(|w| - b_big) computed as sign(w^2 - b_big^2)
                # (w^2 on DVE so ACT only does 2 ops per k-tile)
                nc.vector.tensor_tensor(av[:], w_t[:], w_t[:], Alu.mult)
                nc.scalar.sign(sb[:], w_t[:])
                nc.scalar.sign(av[:], av[:], bias=nbb2[:])
                # wqn = s_big * (R + ss2); the +R alternates ACT/DVE per
                # k-tile to balance both engines through the quantize window
                if kt % 2 == 0:
                    nc.vector.tensor_scalar(av[:], av[:], R, None, Alu.add)
                else:
                    nc.scalar.activation(av[:], av[:],
                                         mybir.ActivationFunctionType.Identity,
                                         rcon[:])
                if kt < 2 * KDR_PAIRS:
                    dst = wq_dr[kt // 2][:, kt % 2, :]
                else:
                    dst = wq_bf[kt - 2 * KDR_PAIRS][:]
                nc.vector.tensor_tensor(dst, sb[:], av[:], Alu.mult)

            for kt in range(8):
                quantize_w(kt)
            fetch_x(0)
            for kt in range(8, 16):
                quantize_w(kt)
            fetch_x(1)
            for kt in range(16, 24):
                quantize_w(kt)
            fetch_x(2)
            for kt in range(24, KT):
                quantize_w(kt)
            fetch_x(3)

            def mm_dr(ps, osb, j, xd, start):
                nc.tensor.matmul(
                    ps[:], wq_dr[j][:, :, osb * 128:(osb + 1) * 128],
                    xd[:, j, :, :], start=start, stop=False, perf_mode=DR)

            def mm_bf(ps, osb, c, xb, stop):
                nc.tensor.matmul(
                    ps[:], wq_bf[c][:, osb * 128:(osb + 1) * 128],
                    xb[:, c, :], start=False, stop=stop)

            def evict(tb, osb, ps):
                o_t = outp.tile([128, TB], dt.bfloat16, tag="ot")
                # out = b_small * psum + bias  (per-partition bias AP)
                nc.vector.tensor_scalar(o_t[:], ps[:], float(b_small),
                                        bias_sb[:, osb:osb + 1],
                                        Alu.mult, Alu.add)
                nc.scalar.dma_start(oT_r[tb, osb], o_t[:])

            # ---- GEMM  psum[o128, t512] += wq[k,o].T @ x[k,t]
            # tb=0 consumes wq tiles in bursts of 8 quantize outputs across
            # its 4 psum groups so the PE starts while wq trickles in.
            # Quantize order: kt 0..19 -> DR pairs 0..9, kt 20..31 -> bf 0..11.
            items = ([("dr", j) for j in range(KDR_PAIRS)]
                     + [("bf", c) for c in range(KBF)])
            bursts = [items[0:4], items[4:8], items[8:14], items[14:22]]
            assert sum(len(b) for b in bursts) == KDR_PAIRS + KBF

            xd0, xb0 = x_tiles.pop(0)
            ps0 = [psp.tile([128, TB], dt.float32, tag="ps", name=f"ps0{osb}")
                   for osb in range(O_SUB)]
            first = [True] * O_SUB
            for burst in bursts:
                # DR items first, then bf items, each across all 4 groups,
                # to minimize PE mode switches within the burst
                for kind_sel in ("dr", "bf"):
                    sel = [it for it in burst if it[0] == kind_sel]
                    if not sel:
                        continue
                    for osb in range(O_SUB):
                        for kind, idx in sel:
                            if kind == "dr":
                                mm_dr(ps0[osb], osb, idx, xd0, first[osb])
                                first[osb] = False
                            else:
                                mm_bf(ps0[osb], osb, idx, xb0,
                                      stop=(idx == KBF - 1))
            for osb in range(O_SUB):
                evict(0, osb, ps0[osb])

            for tb in range(1, N_TB):
                if tb + 2 < N_TB:
                    fetch_x(tb + 2)
                xd, xb = x_tiles.pop(tb)
                ps = [psp.tile([128, TB], dt.float32, tag="ps", name="ps")
                      for _ in range(O_SUB)]
                # DR phase (one mode switch per tb) then bf16 phase
                for osb in range(O_SUB):
                    for j in range(KDR_PAIRS):
                        mm_dr(ps[osb], osb, j, xd, start=(j == 0))
                for osb in range(O_SUB):
                    for c in range(KBF):
                        mm_bf(ps[osb], osb, c, xb, stop=(c == KBF - 1))
                    if tb == N_TB - 1:
                        # last tb: half-column evict/store slices so the
                        # stores overlap the evictions in the drain tail
                        o_t = outp.tile([128, TB], dt.bfloat16, tag="ot",
                                        name="ot_tail")
                        for half in range(2):
                            sl = slice(half * (TB // 2), (half + 1) * (TB // 2))
                            nc.vector.tensor_scalar(
                                o_t[:, sl], ps[osb][:, sl], float(b_small),
                                bias_sb[:, osb:osb + 1], Alu.mult, Alu.add)
                            nc.scalar.dma_start(oT_r[tb, osb][:, sl],
                                                o_t[:, sl])
                    else:
                        evict(tb, osb, ps[osb])

    nc.compile()
    return nc


def kernel(x, weight, bias, basis):
    from concourse import bass_utils

    x = np.asarray(x, dtype=np.float32)
    weight = np.asarray(weight, dtype=np.float32)
    bias = np.asarray(bias, dtype=np.float32)
    basis = np.asarray(basis, dtype=np.float32)

    b_small, b_big = sorted(float(v) for v in np.abs(basis))

    # ---- host-side shard/layout prep (transpose, cast, block)
    # xdr[tb, p, j, i, t] = e4m3(x[tb*512+t, (2j+i)*128+p])   k < KDR
    # xbf[tb, p, c, t]    = bf16(x[tb*512+t, KDR + c*128+p])  k >= KDR
    x8 = x[:, :KDR].astype(ml_dtypes.float8_e4m3)
    xdr = np.ascontiguousarray(
        x8.reshape(N_TB, TB, KDR_PAIRS, 2, 128).transpose(0, 4, 2, 3, 1))
    xb16 = x[:, KDR:].astype(ml_dtypes.bfloat16)
    xbf = np.ascontiguousarray(
        xb16.reshape(N_TB, TB, KBF, 128).transpose(0, 3, 2, 1))
    wt = weight.T                                        # [4096 in, 4096 out]

    in_maps = []
    for c in range(N_CORES):
        wb = np.ascontiguousarray(
            wt[:, c * O_SHARD:(c + 1) * O_SHARD]).reshape(KT, 128, O_SHARD)
        m = {
            "wT": wb,
            "xdr": xdr,
            "xbf": xbf,
            "bs": np.ascontiguousarray(
                bias[c * O_SHARD:(c + 1) * O_SHARD].reshape(O_SUB, 128).T),
        }
        in_maps.append(m)

    nc = _build_nc(b_small, b_big)
    trace = os.environ.get("LQ_TRACE", "") == "1"

    # random-projection ground truth for readback validation: catches
    # transport-level corruption (e.g. output blocks read back before the
    # final stores land). Expectation includes the fp8/bf16 casts of x, so
    # the residual only contains device arithmetic noise (fp32 psum order,
    # DoubleRow per-cell adder rounding ~1e-4).
    rng = np.random.default_rng(12345)
    v = rng.standard_normal(OUT_F)
    wqn = np.sign(weight) * (2.0 + np.sign(weight * weight
                                           - np.float32(b_big) ** 2))
    u = (b_small * wqn.astype(np.float64)).T @ v          # [IN_F]
    r_exp = (x8.astype(np.float64) @ u[:KDR]
             + xb16.astype(np.float64) @ u[KDR:] + float(bias @ v))
    r_scale = np.linalg.norm(r_exp)

    out = None
    for attempt in range(3):
        try:
            res = bass_utils.run_bass_kernel_spmd(
                nc, in_maps, core_ids=list(range(N_CORES)), trace=trace)
        except Exception:
            if attempt == 2:
                raise
            continue

        LAST_RUN_INFO.clear()
        LAST_RUN_INFO["exec_time_ns"] = res.exec_time_ns
        LAST_RUN_INFO["profile_json"] = res.profile_json
        LAST_RUN_INFO["nc"] = nc
        LAST_RUN_INFO["in_maps"] = in_maps

        # oT blocked [tb, osb, p, t]: rows osb*128+p of shard, cols tb*512+t
        outT = np.concatenate(
            [res.results[c]["oT"].transpose(1, 2, 0, 3).reshape(O_SHARD, TOKENS)
             for c in range(N_CORES)], axis=0)
        out = np.ascontiguousarray(outT.T).astype(np.float32)

        resid = np.linalg.norm(out.astype(np.float64) @ v - r_exp) / r_scale
        if resid < 5e-3:
            break
    return out


# revision 11
# speedup vs baseline: 1.4397x; 1.4282x over previous
"""LQLinear (2-bit learned VQ linear) Trainium2 kernel.

Math (Q_T=1): the least-squares basis refit only feeds the *discarded*
buffer update, so the forward output is

    out = x @ wq.T + bias

where wq bucketizes weight into the 4 sorted levels {+-b_small +- b_big}
(b_small, b_big = sorted |basis|), thresholds at midpoints {-b_big, 0, +b_big}.

Device strategy (8 cores, out_features-sharded, 512 rows each):
  - wq = b_small * wqn with wqn in {+-1, +-3} for the reference basis
    (b_big = 2*b_small): EXACT in fp8e4.
  - greedy sign quantization == bucketize, decided in f32: s_big = sign(w),
    ss2 = sign(w^2 - b_big^2), wqn = s_big * (2 + ss2).
  - GEMM: ALL matmuls are fp8e4m3 DoubleRow (256 k rows per MM, ~241ns at
    N=512 = 578 cyc @2.4GHz):
      * 16 main pairs cover all 4096 k with x cast to e4m3 (rel err of
        e4m3-x alone: 2.32e-2, over the 2e-2 gate), plus
      * NHL=5 residual-correction pairs re-running k 0..1279 with moving
        x_c = e4m3(x - e4m3(x)) and the SAME wq stationary tiles, which
        cancels those dims' quantization error (exact host-checked rel
        err 1.922e-2; NHL=6 gives 1.832e-2).
    CRITICAL clock finding: mixing DR and normal-mode matmuls in one NEFF
    drops the WHOLE core clock 2.4 -> 2.0 GHz (every engine slows 1.2x,
    measured), so a bf16 k-split loses to all-DR + redundant correction
    pairs even though corrections redo work. All-DR keeps 2.4 GHz.
    MM floor: 64 groups x 21 x 241ns ~= 324us vs 437us for bf16-only.
  - Quantize is pipelined per k-tile; tb=0 consumes pairs in bursts as they
    emerge (correction MM for pair j issues right after main MM j since the
    stationary tile is shared).
  - w-loads interleave ahead of x tb-fetches in groups of 8 on the SAME
    (sync) HW-DGE ring (separate rings starve the 2KB w packets behind the
    big x packets). Out-stores use the scalar ring.
  - DVE evicts PSUM with fused out = b_small*psum + bias[o].
  - Host prep is layout-only sharding work (transpose/cast/block).

Measured NOT to help in prior sessions: bf16/fp8-DR mixed k-split (406us
— clock throttled), fp8 e3m4 DoubleRow (rejected by walrus
checkMatmultPerfMode), full hi+lo on all k (2x DR work, 494us),
interleaving 2 token blocks across all 8 psum banks, HAM warmup MMs.
"""

import os
import sys

for _p in ("/opt/trn_rl_repo", "/root/.axon_site/_ro/trn_rl_repo"):
    if os.path.isdir(_p) and _p not in sys.path:
        sys.path.insert(0, _p)

import numpy as np
import ml_dtypes

N_CORES = 8
TOKENS = 8192
IN_F = 4096
OUT_F = 4096
O_SHARD = OUT_F // N_CORES          # 512 output rows per core
KT = IN_F // 128                    # 32 k-tiles
NP_MAIN = KT // 2                   # 16 DoubleRow pairs cover all k
TB = 512                            # token block (psum free dim)
N_TB = TOKENS // TB                 # 16 token blocks
O_SUB = O_SHARD // 128              # 4 output subtiles per core

NHL = int(os.environ.get("LQ_NHL", "5"))   # residual-correction pairs
KC = 256 * NHL                      # corrected k-dims

LAST_RUN_INFO = {}


def _build_nc(b_small: float, b_big: float):
    import concourse.mybir as mybir
    import concourse.tile as tile
    from concourse import bacc

    dt = mybir.dt
    Alu = mybir.AluOpType
    DR = mybir.MatmulPerfMode.DoubleRow

    R = b_big / b_small

    nc = bacc.Bacc("TRN2", target_bir_lowering=False,
                   debug=os.environ.get("LQ_DEBUG", "0") == "1")

    # blocked, fully-contiguous-per-partition host layouts
    wT = nc.dram_tensor("wT", [KT, 128, O_SHARD], dt.float32, kind="ExternalInput")
    xdr = nc.dram_tensor("xdr", [N_TB, 128, NP_MAIN, 2, TB], dt.float8e4,
                         kind="ExternalInput")
    xc = None
    if NHL:
        xc = nc.dram_tensor("xc", [N_TB, 128, NHL, 2, TB], dt.float8e4,
                            kind="ExternalInput")
    bs = nc.dram_tensor("bs", [128, O_SUB], dt.float32, kind="ExternalInput")
    oT = nc.dram_tensor("oT", [N_TB, O_SUB, 128, TB], dt.bfloat16,
                        kind="ExternalOutput")

    wT_r = wT.ap()                  # [kt][128, 512]
    xdr_r = xdr.ap()                # [tb][128, 16, 2, 512]
    xc_r = xc.ap() if xc is not None else None  # [tb][128, NHL, 2, 512]
    oT_r = oT.ap()                  # [tb][osb][128, 512]

    with tile.TileContext(nc) as tc:
        with (
            tc.tile_pool(name="const", bufs=1) as const,
            tc.tile_pool(name="wq", bufs=1) as wqp,
            tc.tile_pool(name="wload", bufs=8) as wload,
            tc.tile_pool(name="quant", bufs=4) as qp,
            tc.tile_pool(name="xdrp", bufs=3) as xdrp,
            tc.tile_pool(name="xcp", bufs=3) as xcp,
            tc.tile_pool(name="outp", bufs=8) as outp,
            tc.tile_pool(name="psum", bufs=8, space="PSUM") as psp,
        ):
            bias_sb = const.tile([128, O_SUB], dt.float32)
            nc.sync.dma_start(bias_sb[:], bs.ap())
            nbb2 = const.tile([128, 1], dt.float32, tag="nbb2")
            nc.vector.memset(nbb2[:], -float(np.float32(b_big) * np.float32(b_big)))
            rcon = const.tile([128, 1], dt.float32, tag="rcon")
            nc.vector.memset(rcon[:], R)

            # persistent quantized-weight DR pair tiles (shared by main and
            # correction matmuls)
            wq_dr = [wqp.tile([128, 2, O_SHARD], dt.float8e4, tag=f"wqdr{j}",
                              name=f"wqdr{j}")
                     for j in range(NP_MAIN)]

            x_tiles = {}

            def fetch_x(tb):
                xd = xdrp.tile([128, NP_MAIN, 2, TB], dt.float8e4,
                               tag="xd", name="xd")
                nc.sync.dma_start(xd[:], xdr_r[tb])
                xct = None
                if NHL:
                    xct = xcp.tile([128, NHL, 2, TB], dt.float8e4,
                                   tag="xc", name="xc")
                    nc.sync.dma_start(xct[:], xc_r[tb])
                x_tiles[tb] = (xd, xct)

            # ---- quantize weight shard -> wqn {+-1,+-R} fp8, one tile per kt
            def quantize_w(kt):
                w_t = wload.tile([128, O_SHARD], dt.float32, tag="wl")
                nc.sync.dma_start(w_t[:], wT_r[kt])
                sb = qp.tile([128, O_SHARD], dt.float32, tag="sb")
                av = qp.tile([128, O_SHARD], dt.float32, tag="av")
                # ss2 = sign(|w| - b_big) computed as sign(w^2 - b_big^2)
                # (w^2 on DVE so ACT only does 2 ops per k-tile)
                nc.vector.tensor_tensor(av[:], w_t[:], w_t[:], Alu.mult)
                nc.scalar.sign(sb[:], w_t[:])
                nc.scalar.sign(av[:], av[:], bias=nbb2[:])
                # wqn = s_big * (R + ss2); the +R alternates ACT/DVE per
                # k-tile to balance both engines through the quantize window
                if kt % 2 == 0:
                    nc.vector.tensor_scalar(av[:], av[:], R, None, Alu.add)
                else:
                    nc.scalar.activation(av[:], av[:],
                                         mybir.ActivationFunctionType.Identity,
                                         rcon[:])
                nc.vector.tensor_tensor(wq_dr[kt // 2][:, kt % 2, :],
                                        sb[:], av[:], Alu.mult)

            for kt in range(8):
                quantize_w(kt)
            fetch_x(0)
            for kt in range(8, 16):
                quantize_w(kt)
            fetch_x(1)
            for kt in range(16, 24):
                quantize_w(kt)
            fetch_x(2)
            for kt in range(24, KT):
                quantize_w(kt)
            fetch_x(3)

            def mm(ps, osb, j, x_t, start, stop):
                nc.tensor.matmul(
                    ps[:], wq_dr[j][:, :, osb * 128:(osb + 1) * 128],
                    x_t[:, j, :, :], start=start, stop=stop, perf_mode=DR)

            def evict(tb, osb, ps):
                o_t = outp.tile([128, TB], dt.bfloat16, tag="ot")
                # out = b_small * psum + bias  (per-partition bias AP)
                nc.vector.tensor_scalar(o_t[:], ps[:], float(b_small),
                                        bias_sb[:, osb:osb + 1],
                                        Alu.mult, Alu.add)
                nc.scalar.dma_start(oT_r[tb, osb], o_t[:])

            # ---- GEMM  psum[o128, t512] += wq[k,o].T @ x[k,t], all DR MMs.
            # items: ("m", j) main pair, ("c", j) correction pair (same
            # stationary tile, xc moving). Pair j's weights land at kt 2j+1;
            # its correction can issue right after.
            items = []
            for j in range(NP_MAIN):
                items.append(("m", j))
                if j < NHL:
                    items.append(("c", j))
            n_items = len(items)
            ready_kt = {("m", j): 2 * j + 1 for j in range(NP_MAIN)}
            ready_kt.update({("c", j): 2 * j + 1 for j in range(NHL)})
            last_item = items[-1]

            # tb=0: consume items in bursts of 8 quantized k-tiles across
            # the 4 psum groups so the PE starts while wq trickles in
            bursts = []
            lo = 0
            for kt_end in range(8, KT + 1, 8):
                hi = sum(1 for it in items if ready_kt[it] < kt_end)
                bursts.append(items[lo:hi])
                lo = hi
            assert sum(len(b) for b in bursts) == n_items

            xd0, xc0 = x_tiles.pop(0)
            ps0 = [psp.tile([128, TB], dt.float32, tag="ps", name=f"ps0{osb}")
                   for osb in range(O_SUB)]
            first = [True] * O_SUB
            for burst in bursts:
                for osb in range(O_SUB):
                    for it in burst:
                        kind, j = it
                        mm(ps0[osb], osb, j, xd0 if kind == "m" else xc0,
                           start=first[osb], stop=(it == last_item))
                        first[osb] = False
            for osb in range(O_SUB):
                evict(0, osb, ps0[osb])

            for tb in range(1, N_TB):
                if tb + 2 < N_TB:
                    fetch_x(tb + 2)
                xd, xct = x_tiles.pop(tb)
                for osb in range(O_SUB):
                    ps = psp.tile([128, TB], dt.float32, tag="ps", name="ps")
                    for j in range(NP_MAIN):
                        mm(ps, osb, j, xd, start=(j == 0),
                           stop=(NHL == 0 and j == NP_MAIN - 1))
                    for j in range(NHL):
                        mm(ps, osb, j, xct, start=False, stop=(j == NHL - 1))
                    if tb == N_TB - 1:
                        # last tb: half-column evict/store slices so the
                        # stores overlap the evictions in the drain tail
                        o_t = outp.tile([128, TB], dt.bfloat16, tag="ot",
                                        name="ot_tail")
                        for half in range(2):
                            sl = slice(half * (TB // 2), (half + 1) * (TB // 2))
                            nc.vector.tensor_scalar(
                                o_t[:, sl], ps[:, sl], float(b_small),
                                bias_sb[:, osb:osb + 1], Alu.mult, Alu.add)
                            nc.scalar.dma_start(oT_r[tb, osb][:, sl],
                                                o_t[:, sl])
                    else:
                        evict(tb, osb, ps)

    nc.compile()
    return nc


def kernel(x, weight, bias, basis):
    from concourse import bass_utils

    x = np.asarray(x, dtype=np.float32)
    weight = np.asarray(weight, dtype=np.float32)
    bias = np.asarray(bias, dtype=np.float32)
    basis = np.asarray(basis, dtype=np.float32)

    b_small, b_big = sorted(float(v) for v in np.abs(basis))

    # ---- host-side shard/layout prep (transpose, cast, block)
    # xdr[tb, p, j, i, t] = e4m3(x[tb*512+t, (2j+i)*128+p])
    # xc[tb, p, j, i, t]  = e4m3((x - e4m3(x))[tb*512+t, (2j+i)*128+p]), j<NHL
    f8 = ml_dtypes.float8_e4m3
    x8 = x.astype(f8)
    xdr = np.ascontiguousarray(
        x8.reshape(N_TB, TB, NP_MAIN, 2, 128).transpose(0, 4, 2, 3, 1))
    xlo8 = (x[:, :KC] - x8[:, :KC].astype(np.float32)).astype(f8)
    xcb = np.ascontiguousarray(
        xlo8.reshape(N_TB, TB, NHL, 2, 128).transpose(0, 4, 2, 3, 1)) \
        if NHL else None
    wt = weight.T                                        # [4096 in, 4096 out]

    in_maps = []
    for c in range(N_CORES):
        wb = np.ascontiguousarray(
            wt[:, c * O_SHARD:(c + 1) * O_SHARD]).reshape(KT, 128, O_SHARD)
        m = {
            "wT": wb,
            "xdr": xdr,
            "bs": np.ascontiguousarray(
                bias[c * O_SHARD:(c + 1) * O_SHARD].reshape(O_SUB, 128).T),
        }
        if NHL:
            m["xc"] = xcb
        in_maps.append(m)

    nc = _build_nc(b_small, b_big)
    trace = os.environ.get("LQ_TRACE", "") == "1"

    # random-projection ground truth for readback validation: catches
    # transport-level corruption (e.g. output blocks read back before the
    # final stores land). Expectation includes the fp8 casts of x, so the
    # residual only contains device arithmetic noise (fp32 psum order,
    # DoubleRow per-cell rounding ~1e-4).
    rng = np.random.default_rng(12345)
    v = rng.standard_normal(OUT_F)
    wqn = np.sign(weight) * (2.0 + np.sign(weight * weight
                                           - np.float32(b_big) ** 2))
    u = (b_small * wqn.astype(np.float64)).T @ v          # [IN_F]
    r_exp = x8.astype(np.float64) @ u + float(bias @ v)
    if NHL:
        r_exp = r_exp + xlo8.astype(np.float64) @ u[:KC]
    r_scale = np.linalg.norm(r_exp)

    out = None
    for attempt in range(3):
        try:
            res = bass_utils.run_bass_kernel_spmd(
                nc, in_maps, core_ids=list(range(N_CORES)), trace=trace)
        except Exception:
            if attempt == 2:
                raise
            continue

        LAST_RUN_INFO.clear()
        LAST_RUN_INFO["exec_time_ns"] = res.exec_time_ns
        LAST_RUN_INFO["profile_json"] = res.profile_json
        LAST_RUN_INFO["nc"] = nc
        LAST_RUN_INFO["in_maps"] = in_maps

        # oT blocked [tb, osb, p, t]: rows osb*128+p of shard, cols tb*512+t
        outT = np.concatenate(
            [res.results[c]["oT"].transpose(1, 2, 0, 3).reshape(O_SHARD, TOKENS)
             for c in range(N_CORES)], axis=0)
        out = np.ascontiguousarray(outT.T).astype(np.float32)

        resid = np.linalg.norm(out.astype(np.float64) @ v - r_exp) / r_scale
        if resid < 5e-3:
            break
    return out


# revision 12
# speedup vs baseline: 1.4489x; 1.0064x over previous
"""LQLinear (2-bit learned VQ linear) Trainium2 kernel.

Math (Q_T=1): the least-squares basis refit only feeds the *discarded*
buffer update, so the forward output is

    out = x @ wq.T + bias

where wq bucketizes weight into the 4 sorted levels {+-b_small +- b_big}
(b_small, b_big = sorted |basis|), thresholds at midpoints {-b_big, 0, +b_big}.

Device strategy (8 cores, out_features-sharded, 512 rows each):
  - wq = b_small * wqn with wqn in {+-1, +-3} for the reference basis
    (b_big = 2*b_small): EXACT in fp8e4.
  - greedy sign quantization == bucketize, decided in f32: s_big = sign(w),
    ss2 = sign(w^2 - b_big^2), wqn = s_big * (2 + ss2).
  - GEMM: ALL matmuls are fp8e4m3 DoubleRow (256 k rows per MM, ~241ns at
    N=512 = 578 cyc @2.4GHz):
      * 16 main pairs cover all 4096 k with x cast to e4m3 (rel err of
        e4m3-x alone: 2.32e-2, over the 2e-2 gate), plus
      * NHL=5 residual-correction pairs re-running k 0..1279 with moving
        x_c = e4m3(x - e4m3(x)) and the SAME wq stationary tiles, which
        cancels those dims' quantization error (exact host-checked rel
        err 1.922e-2; NHL=6 gives 1.832e-2).
    CRITICAL clock finding: mixing DR and normal-mode matmuls in one NEFF
    drops the WHOLE core clock 2.4 -> 2.0 GHz (every engine slows 1.2x,
    measured), so a bf16 k-split loses to all-DR + redundant correction
    pairs even though corrections redo work. All-DR keeps 2.4 GHz.
    MM floor: 64 groups x 21 x 241ns ~= 324us vs 437us for bf16-only.
  - Quantize is pipelined per k-tile; tb=0 consumes pairs in bursts as they
    emerge (correction MM for pair j issues right after main MM j since the
    stationary tile is shared).
  - w-loads interleave ahead of x tb-fetches in groups of 8 on the SAME
    (sync) HW-DGE ring (separate rings starve the 2KB w packets behind the
    big x packets). Out-stores use the scalar ring.
  - DVE evicts PSUM with fused out = b_small*psum + bias[o].
  - Host prep is layout-only sharding work (transpose/cast/block).

Measured NOT to help in prior sessions: bf16/fp8-DR mixed k-split (406us
— clock throttled), fp8 e3m4 DoubleRow (rejected by walrus
checkMatmultPerfMode), full hi+lo on all k (2x DR work, 494us),
interleaving 2 token blocks across all 8 psum banks, HAM warmup MMs.
"""

import os
import sys

for _p in ("/opt/trn_rl_repo", "/root/.axon_site/_ro/trn_rl_repo"):
    if os.path.isdir(_p) and _p not in sys.path:
        sys.path.insert(0, _p)

import numpy as np
import ml_dtypes

N_CORES = 8
TOKENS = 8192
IN_F = 4096
OUT_F = 4096
O_SHARD = OUT_F // N_CORES          # 512 output rows per core
KT = IN_F // 128                    # 32 k-tiles
NP_MAIN = KT // 2                   # 16 DoubleRow pairs cover all k
TB = 512                            # token block (psum free dim)
N_TB = TOKENS // TB                 # 16 token blocks
O_SUB = O_SHARD // 128              # 4 output subtiles per core

NHL = int(os.environ.get("LQ_NHL", "5"))   # residual-correction pairs
KC = 256 * NHL                      # corrected k-dims

LAST_RUN_INFO = {}


def _build_nc(b_small: float, b_big: float):
    import concourse.mybir as mybir
    import concourse.tile as tile
    from concourse import bacc

    dt = mybir.dt
    Alu = mybir.AluOpType
    DR = mybir.MatmulPerfMode.DoubleRow

    R = b_big / b_small

    nc = bacc.Bacc("TRN2", target_bir_lowering=False,
                   debug=os.environ.get("LQ_DEBUG", "0") == "1")

    # blocked, fully-contiguous-per-partition host layouts
    wT = nc.dram_tensor("wT", [KT, 128, O_SHARD], dt.float32, kind="ExternalInput")
    xdr = nc.dram_tensor("xdr", [N_TB, 128, NP_MAIN, 2, TB], dt.float8e4,
                         kind="ExternalInput")
    xc = None
    if NHL:
        xc = nc.dram_tensor("xc", [N_TB, 128, NHL, 2, TB], dt.float8e4,
                            kind="ExternalInput")
    bs = nc.dram_tensor("bs", [128, O_SUB], dt.float32, kind="ExternalInput")
    oT = nc.dram_tensor("oT", [N_TB, O_SUB, 128, TB], dt.bfloat16,
                        kind="ExternalOutput")

    wT_r = wT.ap()                  # [kt][128, 512]
    xdr_r = xdr.ap()                # [tb][128, 16, 2, 512]
    xc_r = xc.ap() if xc is not None else None  # [tb][128, NHL, 2, 512]
    oT_r = oT.ap()                  # [tb][osb][128, 512]

    with tile.TileContext(nc) as tc:
        with (
            tc.tile_pool(name="const", bufs=1) as const,
            tc.tile_pool(name="wq", bufs=1) as wqp,
            tc.tile_pool(name="wload", bufs=8) as wload,
            tc.tile_pool(name="quant", bufs=4) as qp,
            tc.tile_pool(name="xdrp", bufs=3) as xdrp,
            tc.tile_pool(name="xcp", bufs=3) as xcp,
            tc.tile_pool(name="outp", bufs=8) as outp,
            tc.tile_pool(name="psum", bufs=8, space="PSUM") as psp,
        ):
            bias_sb = const.tile([128, O_SUB], dt.float32)
            nc.sync.dma_start(bias_sb[:], bs.ap())
            nbb2 = const.tile([128, 1], dt.float32, tag="nbb2")
            nc.vector.memset(nbb2[:], -float(np.float32(b_big) * np.float32(b_big)))
            rcon = const.tile([128, 1], dt.float32, tag="rcon")
            nc.vector.memset(rcon[:], R)

            # persistent quantized-weight DR pair tiles (shared by main and
            # correction matmuls)
            wq_dr = [wqp.tile([128, 2, O_SHARD], dt.float8e4, tag=f"wqdr{j}",
                              name=f"wqdr{j}")
                     for j in range(NP_MAIN)]

            x_tiles = {}

            def fetch_x(tb):
                xd = xdrp.tile([128, NP_MAIN, 2, TB], dt.float8e4,
                               tag="xd", name="xd")
                nc.sync.dma_start(xd[:], xdr_r[tb])
                xct = None
                if NHL:
                    xct = xcp.tile([128, NHL, 2, TB], dt.float8e4,
                                   tag="xc", name="xc")
                    nc.sync.dma_start(xct[:], xc_r[tb])
                x_tiles[tb] = (xd, xct)

            # ---- quantize weight shard -> wqn {+-1,+-R} fp8, one tile per kt
            def quantize_w(kt):
                w_t = wload.tile([128, O_SHARD], dt.float32, tag="wl")
                nc.sync.dma_start(w_t[:], wT_r[kt])
                sb = qp.tile([128, O_SHARD], dt.float32, tag="sb")
                av = qp.tile([128, O_SHARD], dt.float32, tag="av")
                # ss2 = sign(|w| - b_big) computed as sign(w^2 - b_big^2)
                # (w^2 on DVE so ACT only does 2 ops per k-tile)
                nc.vector.tensor_tensor(av[:], w_t[:], w_t[:], Alu.mult)
                nc.scalar.sign(sb[:], w_t[:])
                nc.scalar.sign(av[:], av[:], bias=nbb2[:])
                # wqn = s_big * (R + ss2); the +R alternates ACT/DVE per
                # k-tile to balance both engines through the quantize window
                if kt % 2 == 0:
                    nc.vector.tensor_scalar(av[:], av[:], R, None, Alu.add)
                else:
                    nc.scalar.activation(av[:], av[:],
                                         mybir.ActivationFunctionType.Identity,
                                         rcon[:])
                nc.vector.tensor_tensor(wq_dr[kt // 2][:, kt % 2, :],
                                        sb[:], av[:], Alu.mult)

            # Issue order on the sync ring is FIFO: only x(0) may interleave
            # into the w stream (tb0's first bursts need it); x(1..3) go
            # AFTER the last w tile so tb1 can start ~30us earlier (pairs
            # complete only once every w tile has landed).
            for kt in range(8):
                quantize_w(kt)
            fetch_x(0)
            for kt in range(8, KT):
                quantize_w(kt)
            fetch_x(1)
            fetch_x(2)
            fetch_x(3)

            def mm(ps, osb, j, x_t, start, stop):
                nc.tensor.matmul(
                    ps[:], wq_dr[j][:, :, osb * 128:(osb + 1) * 128],
                    x_t[:, j, :, :], start=start, stop=stop, perf_mode=DR)

            def evict(tb, osb, ps):
                o_t = outp.tile([128, TB], dt.bfloat16, tag="ot")
                # out = b_small * psum + bias  (per-partition bias AP)
                nc.vector.tensor_scalar(o_t[:], ps[:], float(b_small),
                                        bias_sb[:, osb:osb + 1],
                                        Alu.mult, Alu.add)
                nc.scalar.dma_start(oT_r[tb, osb], o_t[:])

            # ---- GEMM  psum[o128, t512] += wq[k,o].T @ x[k,t], all DR MMs.
            # items: ("m", j) main pair, ("c", j) correction pair (same
            # stationary tile, xc moving). Pair j's weights land at kt 2j+1;
            # its correction can issue right after.
            items = []
            for j in range(NP_MAIN):
                items.append(("m", j))
                if j < NHL:
                    items.append(("c", j))
            n_items = len(items)
            ready_kt = {("m", j): 2 * j + 1 for j in range(NP_MAIN)}
            ready_kt.update({("c", j): 2 * j + 1 for j in range(NHL)})
            last_item = items[-1]

            # tb=0: consume items in bursts of 8 quantized k-tiles across
            # the 4 psum groups so the PE starts while wq trickles in
            bursts = []
            lo = 0
            for kt_end in range(8, KT + 1, 8):
                hi = sum(1 for it in items if ready_kt[it] < kt_end)
                bursts.append(items[lo:hi])
                lo = hi
            assert sum(len(b) for b in bursts) == n_items

            xd0, xc0 = x_tiles.pop(0)
            ps0 = [psp.tile([128, TB], dt.float32, tag="ps", name=f"ps0{osb}")
                   for osb in range(O_SUB)]
            first = [True] * O_SUB
            for burst in bursts:
                for osb in range(O_SUB):
                    for it in burst:
                        kind, j = it
                        mm(ps0[osb], osb, j, xd0 if kind == "m" else xc0,
                           start=first[osb], stop=(it == last_item))
                        first[osb] = False
            for osb in range(O_SUB):
                evict(0, osb, ps0[osb])

            for tb in range(1, N_TB):
                if tb + 2 < N_TB:
                    fetch_x(tb + 2)
                xd, xct = x_tiles.pop(tb)
                for osb in range(O_SUB):
                    ps = psp.tile([128, TB], dt.float32, tag="ps", name="ps")
                    for j in range(NP_MAIN):
                        mm(ps, osb, j, xd, start=(j == 0),
                           stop=(NHL == 0 and j == NP_MAIN - 1))
                    for j in range(NHL):
                        mm(ps, osb, j, xct, start=False, stop=(j == NHL - 1))
                    if tb == N_TB - 1:
                        # last tb: half-column evict/store slices so the
                        # stores overlap the evictions in the drain tail
                        o_t = outp.tile([128, TB], dt.bfloat16, tag="ot",
                                        name="ot_tail")
                        for half in range(2):
                            sl = slice(half * (TB // 2), (half + 1) * (TB // 2))
                            nc.vector.tensor_scalar(
                                o_t[:, sl], ps[:, sl], float(b_small),
                                bias_sb[:, osb:osb + 1], Alu.mult, Alu.add)
                            nc.scalar.dma_start(oT_r[tb, osb][:, sl],
                                                o_t[:, sl])
                    else:
                        evict(tb, osb, ps)

    nc.compile()
    return nc


def kernel(x, weight, bias, basis):
    from concourse import bass_utils

    x = np.asarray(x, dtype=np.float32)
    weight = np.asarray(weight, dtype=np.float32)
    bias = np.asarray(bias, dtype=np.float32)
    basis = np.asarray(basis, dtype=np.float32)

    b_small, b_big = sorted(float(v) for v in np.abs(basis))

    # ---- host-side shard/layout prep (transpose, cast, block)
    # xdr[tb, p, j, i, t] = e4m3(x[tb*512+t, (2j+i)*128+p])
    # xc[tb, p, j, i, t]  = e4m3((x - e4m3(x))[tb*512+t, (2j+i)*128+p]), j<NHL
    f8 = ml_dtypes.float8_e4m3
    x8 = x.astype(f8)
    xdr = np.ascontiguousarray(
        x8.reshape(N_TB, TB, NP_MAIN, 2, 128).transpose(0, 4, 2, 3, 1))
    xlo8 = (x[:, :KC] - x8[:, :KC].astype(np.float32)).astype(f8)
    xcb = np.ascontiguousarray(
        xlo8.reshape(N_TB, TB, NHL, 2, 128).transpose(0, 4, 2, 3, 1)) \
        if NHL else None
    wt = weight.T                                        # [4096 in, 4096 out]

    in_maps = []
    for c in range(N_CORES):
        wb = np.ascontiguousarray(
            wt[:, c * O_SHARD:(c + 1) * O_SHARD]).reshape(KT, 128, O_SHARD)
        m = {
            "wT": wb,
            "xdr": xdr,
            "bs": np.ascontiguousarray(
                bias[c * O_SHARD:(c + 1) * O_SHARD].reshape(O_SUB, 128).T),
        }
        if NHL:
            m["xc"] = xcb
        in_maps.append(m)

    nc = _build_nc(b_small, b_big)
    trace = os.environ.get("LQ_TRACE", "") == "1"

    # random-projection ground truth for readback validation: catches
    # transport-level corruption (e.g. output blocks read back before the
    # final stores land). Expectation includes the fp8 casts of x, so the
    # residual only contains device arithmetic noise (fp32 psum order,
    # DoubleRow per-cell rounding ~1e-4).
    rng = np.random.default_rng(12345)
    v = rng.standard_normal(OUT_F)
    wqn = np.sign(weight) * (2.0 + np.sign(weight * weight
                                           - np.float32(b_big) ** 2))
    u = (b_small * wqn.astype(np.float64)).T @ v          # [IN_F]
    r_exp = x8.astype(np.float64) @ u + float(bias @ v)
    if NHL:
        r_exp = r_exp + xlo8.astype(np.float64) @ u[:KC]
    r_scale = np.linalg.norm(r_exp)

    out = None
    for attempt in range(3):
        try:
            res = bass_utils.run_bass_kernel_spmd(
                nc, in_maps, core_ids=list(range(N_CORES)), trace=trace)
        except Exception:
            if attempt == 2:
                raise
            continue

        LAST_RUN_INFO.clear()
        LAST_RUN_INFO["exec_time_ns"] = res.exec_time_ns
        LAST_RUN_INFO["profile_json"] = res.profile_json
        LAST_RUN_INFO["nc"] = nc
        LAST_RUN_INFO["in_maps"] = in_maps

        # oT blocked [tb, osb, p, t]: rows osb*128+p of shard, cols tb*512+t
        outT = np.concatenate(
            [res.results[c]["oT"].transpose(1, 2, 0, 3).reshape(O_SHARD, TOKENS)
             for c in range(N_CORES)], axis=0)
        out = np.ascontiguousarray(outT.T).astype(np.float32)

        resid = np.linalg.norm(out.astype(np.float64) @ v - r_exp) / r_scale
        if resid < 5e-3:
            break
    return out
